# revision 15
# baseline (speedup 1.0000x reference)
"""7x7 'same' 2D convolution over [128, 512, 512] f32, data-parallel on 8 NeuronCores.

Formulation: for each output-row block of M=122 rows, the row-direction
(u-tap) contraction is a banded Toeplitz matmul on the TensorEngine:
    out[i0+m, j] = sum_v sum_r T_v[r, m] * xpad[i0+r, j+v]
with T_v[r, m] = w[r-m, v] (band 0 <= r-m < 7). The 7 column taps (v)
are 7 matmuls accumulating into the same PSUM bank, each reading the
same SBUF x-tile at a shifted column offset. Inputs are cast to fp16
host-side (full-rate on the PE, ~1e-3 rel err); accumulation is fp32;
outputs are stored as bf16 and upcast on the host.

DMA strategy: one staging tile per image holds the 5 overlapping
128-row chunks (row stride 122 between chunks) so each image loads with
2 big DMAs; outputs collect into one [128,5,512] tile and store with 2
DMAs per image into a (p, chunk)-major DRAM layout so the writes walk
DRAM sequentially (strided interleaved writes measured ~25x slower).
The host un-permutes the chunked output. Loads and stores alternate
between the two HWDGE rings (sync / scalar) per image.
"""

import numpy as np

B, H, W = 128, 512, 512
KS = 7
PAD = (KS - 1) // 2          # 3
HP = H + 2 * PAD             # 518
N_CORES = 8
PER_CORE = B // N_CORES      # 16
MBLK = 128 - (KS - 1)        # 122 output rows per full block
NBLK = 5                     # ceil(512 / 122); last block has 24 rows
MRUNT = H - 4 * MBLK         # 24
KRUNT = HP - 4 * MBLK        # 30


def _build_program():
    import concourse.bass as bass
    import concourse.tile as tile
    from concourse import bacc, mybir

    f16 = mybir.dt.float16
    bf16 = mybir.dt.bfloat16
    f32 = mybir.dt.float32

    nc = bacc.Bacc("TRN2", target_bir_lowering=False, debug=False,
                   num_devices=N_CORES)
    x_ext = nc.declare_dram_parameter("x", [PER_CORE, HP, HP], f16,
                                      isOutput=False)
    t_ext = nc.declare_dram_parameter("toep", [128, KS * 128], f16,
                                      isOutput=False)
    # chunk-major output: opc[img, c, p, :] = out row 122c + p, so each
    # per-block store walks DRAM sequentially.
    opc_ext = nc.declare_dram_parameter("opc", [PER_CORE, 4, MBLK, W],
                                        bf16, isOutput=True)
    ort_ext = nc.declare_dram_parameter("ort", [PER_CORE, MRUNT, W],
                                        bf16, isOutput=True)

    with tile.TileContext(nc) as tc:
        with (
            tc.tile_pool(name="toep", bufs=1) as toep_pool,
            tc.tile_pool(name="xin", bufs=4) as x_pool,
            tc.tile_pool(name="psum", bufs=8, space="PSUM") as psum_pool,
            tc.tile_pool(name="outs", bufs=6) as out_pool,
        ):
            toep_sb = toep_pool.tile([128, KS * 128], f16)
            nc.sync.dma_start(out=toep_sb[:], in_=t_ext[:])

            for img in range(PER_CORE):
                # Stage the image as 5 overlapping 128-row chunks:
                # chunk c holds padded rows [122c, 122c+128).
                stage = x_pool.tile([128, NBLK, HP], f16)
                src = bass.AP(
                    x_ext,
                    img * HP * HP,
                    [(HP, 128), (MBLK * HP, 4), (1, HP)],
                )
                nc.sync.dma_start(out=stage[:, 0:4, :], in_=src)
                # chunk 4: padded rows [488, 518) (30 rows; rest unused)
                nc.sync.dma_start(out=stage[:KRUNT, 4, :],
                                  in_=x_ext[img, 4 * MBLK:HP, :])

                psums = [psum_pool.tile([128, W], f32, name=f"acc{b}",
                                        tag="acc") for b in range(NBLK)]
                # v-outer: all 5 blocks share one Toeplitz per tap, so the
                # stationary operand only changes every 5th matmul.
                for v in range(KS):
                    for b in range(NBLK):
                        kin = 128 if b < 4 else KRUNT
                        nc.tensor.matmul(
                            psums[b][:128, :],
                            toep_sb[:kin, v * 128:(v + 1) * 128],
                            stage[:kin, b, v:v + W],
                            start=(v == 0),
                            stop=(v == KS - 1),
                        )
                for b in range(NBLK):
                    m = MBLK if b < 4 else MRUNT
                    o_sb = out_pool.tile([128, W], bf16, name=f"o{b}",
                                         tag="osb")
                    nc.vector.tensor_copy(o_sb[:m, :], psums[b][:m, :])
                    dst = opc_ext[img, b] if b < 4 else ort_ext[img]
                    nc.scalar.dma_start(out=dst, in_=o_sb[:m, :])
    nc.finalize()
    return nc


def _host_prep(x, w):
    x = np.asarray(x, dtype=np.float32)
    w = np.asarray(w, dtype=np.float32)
    xpad = np.zeros((B, HP, HP), dtype=np.float16)
    xpad[:, PAD:PAD + H, PAD:PAD + W] = x
    # Toeplitz padded to 128 cols (cols >= MBLK are zero -> garbage out
    # rows that are never stored); 128 weight cols also enables FWL.
    toep = np.zeros((128, KS * 128), dtype=np.float16)
    w16 = w.astype(np.float16)
    idx = np.arange(MBLK)
    for v in range(KS):
        for d in range(KS):
            toep[idx + d, v * 128 + idx] = w16[d, v]
    return xpad, toep


def _execute(x, w, **run_kwargs):
    from concourse.bass_utils import run_bass_kernel_spmd

    xpad, toep = _host_prep(x, w)
    nc = _build_program()
    in_maps = [
        {"x": xpad[c * PER_CORE:(c + 1) * PER_CORE], "toep": toep}
        for c in range(N_CORES)
    ]
    res = run_bass_kernel_spmd(nc, in_maps, core_ids=list(range(N_CORES)),
                               **run_kwargs)
    out = np.empty((B, H, W), dtype=np.float32)
    for c in range(N_CORES):
        sl = slice(c * PER_CORE, (c + 1) * PER_CORE)
        opc = np.asarray(res.results[c]["opc"], dtype=np.float32)
        ort = np.asarray(res.results[c]["ort"], dtype=np.float32)
        # opc[img, c, p, :] -> rows (c*MBLK + p)
        out[sl, :4 * MBLK, :] = opc.reshape(PER_CORE, 4 * MBLK, W)
        out[sl, 4 * MBLK:, :] = ort
    return out, res


def kernel(x, w):
    out, _ = _execute(x, w)
    return out


# revision 20
# speedup vs baseline: 1.9888x; 1.9888x over previous
"""7x7 'same' 2D convolution over [128, 512, 512] f32, data-parallel on 8 NeuronCores.

Banded-Toeplitz formulation on the TensorEngine with 32x32 array
packing: the PE array is addressed as 16 independent 32x32 tiles
(tile_position=(32r, 32g)), each computing a 26-row output block
    out[i0+m, j] = sum_v sum_{r'} T_v[r', m] * xpad[i0+r', j+v]
with T_v[r', m] = w[r'-m, v] (band, r'<32, m<26). The 7 column taps (v)
accumulate into PSUM; tile (r, g) writes PSUM bank r, partitions
[32g, 32g+26). A 16-tile tap sweep covers 416 output rows in ~one
N=512 matmul time. Rows 416..511 of four consecutive images are
batched into one 16-tile "tail" group (tile (j, r) <- image j).

Inputs are cast to fp16 and pre-staged host-side into the SBUF slab
layout (partition 32r+p, slab s, col) = padded row 104s + 26r + p, so
each image loads with a single contiguous 663KB DMA. Accumulation is
fp32; outputs are stored as raw bf16 PSUM-bank dumps (6 garbage rows
per 32-row strip) and un-permuted on the host. Stores alternate
between the scalar HWDGE ring and the gpsimd SWDGE ring; loads own
the sync ring.
"""

import numpy as np

B, H, W = 128, 512, 512
KS = 7
PAD = (KS - 1) // 2          # 3
HP = H + 2 * PAD             # 518
N_CORES = 8
PER_CORE = B // N_CORES      # 16
TS = 26                      # output rows per 32x32 tile (32 - 6)
NS = 5                       # slabs per image (4 group-1 + 1 tail)


def _build_program():
    import concourse.bass as bass
    import concourse.tile as tile
    from concourse import bacc, mybir

    f16 = mybir.dt.float16
    bf16 = mybir.dt.bfloat16
    f32 = mybir.dt.float32

    nc = bacc.Bacc("TRN2", target_bir_lowering=False, debug=False,
                   num_devices=N_CORES)
    x_ext = nc.declare_dram_parameter("x", [PER_CORE, 128, NS * HP], f16,
                                      isOutput=False)
    t_ext = nc.declare_dram_parameter("toep", [128, KS * TS], f16,
                                      isOutput=False)
    # og[img, r] = raw dump of PSUM bank r (group 1):
    #   row 32g+p  ->  out row 104g + 26r + p   (valid p < 26)
    og_ext = nc.declare_dram_parameter("og", [PER_CORE, 4, 128, W],
                                       bf16, isOutput=True)
    # ot[tg, r] = tail bank dump: row 32j+p -> img 4tg+j,
    #   out row 416 + 26r + p  (valid p < 26; r=3: p < 18)
    ot_ext = nc.declare_dram_parameter("ot", [PER_CORE // 4, 4, 128, W],
                                       bf16, isOutput=True)

    with tile.TileContext(nc) as tc:
        with (
            tc.tile_pool(name="toep", bufs=1) as toep_pool,
            tc.tile_pool(name="xin", bufs=6) as x_pool,
            tc.tile_pool(name="psum", bufs=8, space="PSUM") as psum_pool,
            tc.tile_pool(name="outs", bufs=8) as out_pool,
        ):
            toep_sb = toep_pool.tile([128, KS * TS], f16)
            nc.sync.dma_start(out=toep_sb[:], in_=t_ext[:])

            def evac(ps, dst, ring):
                o_sb = out_pool.tile([128, W], bf16, name="o", tag="osb")
                nc.vector.tensor_copy(o_sb[:], ps[:])
                ring.dma_start(out=dst, in_=o_sb[:])

            stages = {}
            for img in range(PER_CORE):
                # (partition 32r+p, slab s) = padded row 104s + 26r + p
                stage = x_pool.tile([128, NS * HP], f16, name="stage",
                                    tag="stage")
                nc.sync.dma_start(out=stage[:], in_=x_ext[img])
                stages[img] = stage

                ps = [psum_pool.tile([128, W], f32, name=f"ps{r}",
                                     tag="acc") for r in range(4)]
                for v in range(KS):
                    for g in range(4):
                        for r in range(4):
                            nc.tensor.matmul(
                                ps[r][32 * g:32 * g + TS, :],
                                toep_sb[32 * r:32 * r + 32,
                                        TS * v:TS * (v + 1)],
                                stage[32 * r:32 * r + 32,
                                      g * HP + v:g * HP + v + W],
                                start=(v == 0),
                                stop=(v == KS - 1),
                                tile_position=(32 * r, 32 * g),
                            )
                for r in range(4):
                    evac(ps[r], og_ext[img, r],
                         nc.scalar if r % 2 == 0 else nc.gpsimd)

                if img % 4 == 3:
                    tg = img // 4
                    pst = [psum_pool.tile([128, W], f32, name=f"pt{r}",
                                          tag="acc") for r in range(4)]
                    for v in range(KS):
                        for j in range(4):
                            st_j = stages[4 * tg + j]
                            for r in range(4):
                                kin = 32 if r < 3 else 24
                                mm = TS if r < 3 else 18
                                nc.tensor.matmul(
                                    pst[r][32 * j:32 * j + mm, :],
                                    toep_sb[32 * r:32 * r + kin,
                                            TS * v:TS * v + mm],
                                    st_j[32 * r:32 * r + kin,
                                         4 * HP + v:4 * HP + v + W],
                                    start=(v == 0),
                                    stop=(v == KS - 1),
                                    tile_position=(32 * r, 32 * j),
                                )
                    for r in range(4):
                        evac(pst[r], ot_ext[tg, r],
                             nc.scalar if r % 2 == 0 else nc.gpsimd)
                    stages = {}
    nc.finalize()
    return nc


def _host_prep(x, w):
    x = np.asarray(x, dtype=np.float32)
    w = np.asarray(w, dtype=np.float32)
    # padded images, with extra zero rows so slab-4/strip-3 reads land
    # on zeros (row indices up to 104*4 + 26*3 + 31 = 525)
    xpad = np.zeros((B, 526, HP), dtype=np.float16)
    xpad[:, PAD:PAD + H, PAD:PAD + W] = x
    # slab layout: (p, s) -> padded row 104s + 26*(p//32) + p%32
    p = np.arange(128)
    s = np.arange(NS)
    ridx = 104 * s[None, :] + 26 * (p[:, None] // 32) + (p[:, None] % 32)
    xslab = xpad[:, ridx, :]                     # [B, 128, NS, HP]
    xslab = np.ascontiguousarray(
        xslab.reshape(B, 128, NS * HP))
    # Toeplitz band [32, 26] per tap, replicated on all 4 partition strips
    toep = np.zeros((128, KS * TS), dtype=np.float16)
    w16 = w.astype(np.float16)
    idx = np.arange(TS)
    for st in range(4):
        for v in range(KS):
            for d in range(KS):
                toep[32 * st + idx + d, TS * v + idx] = w16[d, v]
    return xslab, toep


def _execute(x, w, **run_kwargs):
    from concourse.bass_utils import run_bass_kernel_spmd

    xslab, toep = _host_prep(x, w)
    nc = _build_program()
    in_maps = [
        {"x": xslab[c * PER_CORE:(c + 1) * PER_CORE], "toep": toep}
        for c in range(N_CORES)
    ]
    res = run_bass_kernel_spmd(nc, in_maps, core_ids=list(range(N_CORES)),
                               **run_kwargs)
    out = np.empty((B, H, W), dtype=np.float32)
    for c in range(N_CORES):
        sl = slice(c * PER_CORE, (c + 1) * PER_CORE)
        og = np.asarray(res.results[c]["og"], dtype=np.float32)
        ot = np.asarray(res.results[c]["ot"], dtype=np.float32)
        og5 = og.reshape(PER_CORE, 4, 4, 32, W)[:, :, :, :TS, :]
        # row = 104g + 26r + p  ->  order (g, r, p)
        out[sl, :16 * TS, :] = og5.transpose(0, 2, 1, 3, 4).reshape(
            PER_CORE, 16 * TS, W)
        ot6 = ot.reshape(PER_CORE // 4, 4, 4, 32, W)
        full = ot6[:, :3, :, :TS, :]          # [tg, r<3, j, p<26, w]
        runt = ot6[:, 3:, :, :18, :]          # [tg, r=3, j, p<18, w]
        # rows 416 + 26r + p, per image j
        tail = np.concatenate(
            [full.transpose(0, 2, 1, 3, 4).reshape(PER_CORE // 4, 4, 78, W),
             runt.transpose(0, 2, 1, 3, 4).reshape(PER_CORE // 4, 4, 18, W)],
            axis=2,
        )
        out[sl, 16 * TS:, :] = tail.reshape(PER_CORE, 96, W)
    return out, res


def kernel(x, w):
    out, _ = _execute(x, w)
    return out


# revision 22
# speedup vs baseline: 2.4289x; 1.2213x over previous
"""7x7 'same' 2D convolution over [128, 512, 512] f32, data-parallel on 8 NeuronCores.

Banded-Toeplitz formulation on the TensorEngine with 64x64 array
packing: the PE array runs as 4 independent 64x64 tiles
(tile_position=(64r, 64g)), each computing a 58-row output block
    out[i0+m, j] = sum_v sum_{r'} T_v[r', m] * xpad[i0+r', j+v]
with T_v[r', m] = w[r'-m, v] (band, r'<64, m<58). The 7 column taps (v)
accumulate into PSUM; tile (s, r2, g2) covers out rows
232s + 116g2 + 58r2 + [0,58) and writes PSUM bank (s, r2), partitions
[64g2, 64g2+58). One 4-tile tap sweep streams in one N=512 matmul
time; 8 tiles cover rows 0..463 of an image. Rows 464..511 of four
consecutive images batch into one 4-tile "tail" group.

Inputs are cast to fp16 and pre-staged host-side into the SBUF slab
layout (partition 64r+p, slab q, col) = padded row 116q + 58r + p
(slab 4 = tail rows, duplicated on both strips), so each image loads
with a single contiguous 663KB DMA. Accumulation is fp32; outputs are
stored as raw bf16 PSUM-bank dumps and un-permuted on the host.
PSUM evacuation alternates VectorE / ScalarE; stores alternate the
scalar HWDGE ring and the gpsimd SWDGE ring; loads own the sync ring.
"""

import numpy as np

B, H, W = 128, 512, 512
KS = 7
PAD = (KS - 1) // 2          # 3
HP = H + 2 * PAD             # 518
N_CORES = 8
PER_CORE = B // N_CORES      # 16
TS = 58                      # output rows per 64x64 tile (64 - 6)
NS = 5                       # slabs per image (4 group-1 + 1 tail)
TAILM = H - 8 * TS           # 48 tail output rows per image
TAILK = TAILM + KS - 1       # 54


def _build_program():
    import concourse.bass as bass
    import concourse.tile as tile
    from concourse import bacc, mybir

    f16 = mybir.dt.float16
    bf16 = mybir.dt.bfloat16
    f32 = mybir.dt.float32

    nc = bacc.Bacc("TRN2", target_bir_lowering=False, debug=False,
                   num_devices=N_CORES)
    x_ext = nc.declare_dram_parameter("x", [PER_CORE, 128, NS * HP], f16,
                                      isOutput=False)
    t_ext = nc.declare_dram_parameter("toep", [128, KS * TS], f16,
                                      isOutput=False)
    # og[img, s, r] = dump of PSUM bank (s, r):
    #   row 64g+p  ->  out row 232s + 116g + 58r + p   (valid p < 58)
    og_ext = nc.declare_dram_parameter("og", [PER_CORE, 2, 2, 128, W],
                                       bf16, isOutput=True)
    # ot[tg, r] = tail bank dump: row 64g+p -> img 4tg + 2g + r,
    #   out row 464 + p  (valid p < 48)
    ot_ext = nc.declare_dram_parameter("ot", [PER_CORE // 4, 2, 128, W],
                                       bf16, isOutput=True)

    with tile.TileContext(nc) as tc:
        with (
            tc.tile_pool(name="toep", bufs=1) as toep_pool,
            tc.tile_pool(name="xin", bufs=6) as x_pool,
            tc.tile_pool(name="psum", bufs=8, space="PSUM") as psum_pool,
            tc.tile_pool(name="outs", bufs=8) as out_pool,
        ):
            toep_sb = toep_pool.tile([128, KS * TS], f16)
            nc.sync.dma_start(out=toep_sb[:], in_=t_ext[:])

            def evac(ps, dst, idx):
                o_sb = out_pool.tile([128, W], bf16, name="o", tag="osb")
                if idx % 2 == 0:
                    nc.vector.tensor_copy(o_sb[:], ps[:])
                else:
                    nc.scalar.copy(o_sb[:], ps[:])
                ring = nc.scalar if idx % 4 < 2 else nc.gpsimd
                ring.dma_start(out=dst, in_=o_sb[:])

            stages = {}
            for img in range(PER_CORE):
                # (partition 64r+p, slab q) = padded row 116q + 58r + p
                stage = x_pool.tile([128, NS * HP], f16, name="stage",
                                    tag="stage")
                nc.sync.dma_start(out=stage[:], in_=x_ext[img])
                stages[img] = stage

                ps = [psum_pool.tile([128, W], f32, name=f"ps{i}",
                                     tag="acc") for i in range(4)]
                for v in range(KS):
                    for s in range(2):
                        for g in range(2):
                            for r in range(2):
                                q = 2 * s + g
                                nc.tensor.matmul(
                                    ps[2 * s + r][64 * g:64 * g + TS, :],
                                    toep_sb[64 * r:64 * r + 64,
                                            TS * v:TS * (v + 1)],
                                    stage[64 * r:64 * r + 64,
                                          q * HP + v:q * HP + v + W],
                                    start=(v == 0),
                                    stop=(v == KS - 1),
                                    tile_position=(64 * r, 64 * g),
                                )
                for i in range(4):
                    evac(ps[i], og_ext[img, i // 2, i % 2], img * 4 + i)

                if img % 4 == 3:
                    tg = img // 4
                    pst = [psum_pool.tile([128, W], f32, name=f"pt{r}",
                                          tag="acc") for r in range(2)]
                    for v in range(KS):
                        for j in range(4):
                            r, g = j % 2, j // 2
                            nc.tensor.matmul(
                                pst[r][64 * g:64 * g + TAILM, :],
                                toep_sb[64 * r:64 * r + TAILK,
                                        TS * v:TS * v + TAILM],
                                stages[4 * tg + j][64 * r:64 * r + TAILK,
                                                   4 * HP + v:4 * HP + v + W],
                                start=(v == 0),
                                stop=(v == KS - 1),
                                tile_position=(64 * r, 64 * g),
                            )
                    for r in range(2):
                        evac(pst[r], ot_ext[tg, r], img * 4 + r)
                    stages = {}
    nc.finalize()
    return nc


def _host_prep(x, w):
    x = np.asarray(x, dtype=np.float32)
    w = np.asarray(w, dtype=np.float32)
    # padded images with extra zero rows (slab-4 strip-1 reads to 585)
    xpad = np.zeros((B, 586, HP), dtype=np.float16)
    xpad[:, PAD:PAD + H, PAD:PAD + W] = x
    # slab layout: (p, q) -> padded row 116q + 58*(p//64) + p%64;
    # slab 4 = tail rows 464+, duplicated on both 64-row strips
    p = np.arange(128)
    q = np.arange(NS)
    ridx = 116 * q[None, :] + 58 * (p[:, None] // 64) + (p[:, None] % 64)
    ridx[:, 4] = 464 + (p % 64)
    xslab = np.ascontiguousarray(
        xpad[:, ridx, :].reshape(B, 128, NS * HP))
    # Toeplitz band [64, 58] per tap, replicated on both partition strips
    toep = np.zeros((128, KS * TS), dtype=np.float16)
    w16 = w.astype(np.float16)
    idx = np.arange(TS)
    for st in range(2):
        for v in range(KS):
            for d in range(KS):
                toep[64 * st + idx + d, TS * v + idx] = w16[d, v]
    return xslab, toep


def _execute(x, w, **run_kwargs):
    from concourse.bass_utils import run_bass_kernel_spmd

    xslab, toep = _host_prep(x, w)
    nc = _build_program()
    in_maps = [
        {"x": xslab[c * PER_CORE:(c + 1) * PER_CORE], "toep": toep}
        for c in range(N_CORES)
    ]
    last_err = None
    for _attempt in range(3):
        try:
            res = run_bass_kernel_spmd(nc, in_maps,
                                       core_ids=list(range(N_CORES)),
                                       **run_kwargs)
            break
        except Exception as e:  # transient NRT execute flakes -> retry
            last_err = e
    else:
        raise last_err
    out = np.empty((B, H, W), dtype=np.float32)
    for c in range(N_CORES):
        sl = slice(c * PER_CORE, (c + 1) * PER_CORE)
        og = np.asarray(res.results[c]["og"], dtype=np.float32)
        ot = np.asarray(res.results[c]["ot"], dtype=np.float32)
        og6 = og.reshape(PER_CORE, 2, 2, 2, 64, W)[:, :, :, :, :TS, :]
        # [img, s, r, g, p, w] -> row = 232s + 116g + 58r + p
        out[sl, :8 * TS, :] = og6.transpose(0, 1, 3, 2, 4, 5).reshape(
            PER_CORE, 8 * TS, W)
        ot5 = ot.reshape(PER_CORE // 4, 2, 2, 64, W)[:, :, :, :TAILM, :]
        # [tg, r, g, p, w] -> img 4tg + 2g + r, row 464 + p
        out[sl, 8 * TS:, :] = ot5.transpose(0, 2, 1, 3, 4).reshape(
            PER_CORE, TAILM, W)
    return out, res


def kernel(x, w):
    out, _ = _execute(x, w)
    return out


# revision 24
# speedup vs baseline: 2.5643x; 1.0557x over previous
"""7x7 'same' 2D convolution over [128, 512, 512] f32, data-parallel on 8 NeuronCores.

Banded-Toeplitz formulation on the TensorEngine with 64x64 array
packing: the PE array runs as 4 independent 64x64 tiles
(tile_position=(64r, 64g)), each computing a 58-row output block
    out[i0+m, j] = sum_v sum_{r'} T_v[r', m] * xpad[i0+r', j+v]
with T_v[r', m] = w[r'-m, v] (band, r'<64, m<58). The 7 column taps (v)
accumulate into PSUM; tile (s, r2, g2) covers out rows
232s + 116g2 + 58r2 + [0,58) and writes PSUM bank (s, r2), partitions
[64g2, 64g2+58). One 4-tile tap sweep streams in one N=512 matmul
time; 8 tiles cover rows 0..463 of an image. Rows 464..511 of four
consecutive images batch into one 4-tile "tail" group.

Inputs are cast to fp16 and pre-staged host-side into the SBUF slab
layout (partition 64r+p, slab q, col) = padded row 116q + 58r + p
(slab 4 = tail rows, duplicated on both strips), so each image loads
with a single contiguous 663KB DMA. Accumulation is fp32; outputs are
stored as raw bf16 PSUM-bank dumps and un-permuted on the host.
PSUM evacuation alternates VectorE / ScalarE; stores alternate the
scalar HWDGE ring and the gpsimd SWDGE ring; loads own the sync ring.
"""

import numpy as np

B, H, W = 128, 512, 512
KS = 7
PAD = (KS - 1) // 2          # 3
HP = H + 2 * PAD             # 518
N_CORES = 8
PER_CORE = B // N_CORES      # 16
TS = 58                      # output rows per 64x64 tile (64 - 6)
NS = 5                       # slabs per image (4 group-1 + 1 tail)
TAILM = H - 8 * TS           # 48 tail output rows per image
TAILK = TAILM + KS - 1       # 54


def _build_program():
    import concourse.bass as bass
    import concourse.tile as tile
    from concourse import bacc, mybir

    f16 = mybir.dt.float16
    bf16 = mybir.dt.bfloat16
    f32 = mybir.dt.float32

    nc = bacc.Bacc("TRN2", target_bir_lowering=False, debug=False,
                   num_devices=N_CORES)
    x_ext = nc.declare_dram_parameter("x", [PER_CORE, 128, NS * HP], f16,
                                      isOutput=False)
    t_ext = nc.declare_dram_parameter("toep", [128, KS * TS], f16,
                                      isOutput=False)
    # og[img, s, r] = dump of PSUM bank (s, r):
    #   row 64g+p  ->  out row 232s + 116g + 58r + p   (valid p < 58)
    og_ext = nc.declare_dram_parameter("og", [PER_CORE, 2, 2, 128, W],
                                       bf16, isOutput=True)
    # ot[tg, r] = tail bank dump: row 64g+p -> img 4tg + 2g + r,
    #   out row 464 + p  (valid p < 48)
    ot_ext = nc.declare_dram_parameter("ot", [PER_CORE // 4, 2, 128, W],
                                       bf16, isOutput=True)

    with tile.TileContext(nc) as tc:
        with (
            tc.tile_pool(name="toep", bufs=1) as toep_pool,
            tc.tile_pool(name="xin", bufs=6) as x_pool,
            tc.tile_pool(name="psum", bufs=8, space="PSUM") as psum_pool,
            tc.tile_pool(name="outs", bufs=8) as out_pool,
        ):
            toep_sb = toep_pool.tile([128, KS * TS], f16)
            nc.scalar.dma_start(out=toep_sb[:], in_=t_ext[:])

            def evac(ps, dst, idx):
                o_sb = out_pool.tile([128, W], bf16, name="o", tag="osb")
                if idx % 2 == 0:
                    nc.vector.tensor_copy(o_sb[:], ps[:])
                else:
                    nc.scalar.copy(o_sb[:], ps[:])
                ring = nc.scalar if idx % 4 < 2 else nc.gpsimd
                ring.dma_start(out=dst, in_=o_sb[:])

            stages = {}
            for img in range(PER_CORE):
                # (partition 64r+p, slab q) = padded row 116q + 58r + p
                stage = x_pool.tile([128, NS * HP], f16, name="stage",
                                    tag="stage")
                nc.sync.dma_start(out=stage[:], in_=x_ext[img])
                stages[img] = stage

                # s-groups sequential: only 2 PSUM banks live per group,
                # so allocation never stalls on evacuation of 4 banks.
                for s in range(2):
                    ps = [psum_pool.tile([128, W], f32, name=f"ps{r}",
                                         tag="acc") for r in range(2)]
                    for v in range(KS):
                        for g in range(2):
                            for r in range(2):
                                q = 2 * s + g
                                nc.tensor.matmul(
                                    ps[r][64 * g:64 * g + TS, :],
                                    toep_sb[64 * r:64 * r + 64,
                                            TS * v:TS * (v + 1)],
                                    stage[64 * r:64 * r + 64,
                                          q * HP + v:q * HP + v + W],
                                    start=(v == 0),
                                    stop=(v == KS - 1),
                                    tile_position=(64 * r, 64 * g),
                                )
                    for r in range(2):
                        evac(ps[r], og_ext[img, s, r], img * 4 + 2 * s + r)

                if img % 4 == 3:
                    tg = img // 4
                    pst = [psum_pool.tile([128, W], f32, name=f"pt{r}",
                                          tag="acc") for r in range(2)]
                    for v in range(KS):
                        for j in range(4):
                            r, g = j % 2, j // 2
                            nc.tensor.matmul(
                                pst[r][64 * g:64 * g + TAILM, :],
                                toep_sb[64 * r:64 * r + TAILK,
                                        TS * v:TS * v + TAILM],
                                stages[4 * tg + j][64 * r:64 * r + TAILK,
                                                   4 * HP + v:4 * HP + v + W],
                                start=(v == 0),
                                stop=(v == KS - 1),
                                tile_position=(64 * r, 64 * g),
                            )
                    for r in range(2):
                        evac(pst[r], ot_ext[tg, r], img * 4 + r)
                    stages = {}
    nc.finalize()
    return nc


def _host_prep(x, w):
    x = np.asarray(x, dtype=np.float32)
    w = np.asarray(w, dtype=np.float32)
    # padded images with extra zero rows (slab-4 strip-1 reads to 585)
    xpad = np.zeros((B, 586, HP), dtype=np.float16)
    xpad[:, PAD:PAD + H, PAD:PAD + W] = x
    # slab layout: (p, q) -> padded row 116q + 58*(p//64) + p%64;
    # slab 4 = tail rows 464+, duplicated on both 64-row strips
    p = np.arange(128)
    q = np.arange(NS)
    ridx = 116 * q[None, :] + 58 * (p[:, None] // 64) + (p[:, None] % 64)
    ridx[:, 4] = 464 + (p % 64)
    xslab = np.ascontiguousarray(
        xpad[:, ridx, :].reshape(B, 128, NS * HP))
    # Toeplitz band [64, 58] per tap, replicated on both partition strips
    toep = np.zeros((128, KS * TS), dtype=np.float16)
    w16 = w.astype(np.float16)
    idx = np.arange(TS)
    for st in range(2):
        for v in range(KS):
            for d in range(KS):
                toep[64 * st + idx + d, TS * v + idx] = w16[d, v]
    return xslab, toep


def _execute(x, w, **run_kwargs):
    from concourse.bass_utils import run_bass_kernel_spmd

    xslab, toep = _host_prep(x, w)
    nc = _build_program()
    in_maps = [
        {"x": xslab[c * PER_CORE:(c + 1) * PER_CORE], "toep": toep}
        for c in range(N_CORES)
    ]
    last_err = None
    for _attempt in range(3):
        try:
            res = run_bass_kernel_spmd(nc, in_maps,
                                       core_ids=list(range(N_CORES)),
                                       **run_kwargs)
            break
        except Exception as e:  # transient NRT execute flakes -> retry
            last_err = e
    else:
        raise last_err
    out = np.empty((B, H, W), dtype=np.float32)
    for c in range(N_CORES):
        sl = slice(c * PER_CORE, (c + 1) * PER_CORE)
        og = np.asarray(res.results[c]["og"], dtype=np.float32)
        ot = np.asarray(res.results[c]["ot"], dtype=np.float32)
        og6 = og.reshape(PER_CORE, 2, 2, 2, 64, W)[:, :, :, :, :TS, :]
        # [img, s, r, g, p, w] -> row = 232s + 116g + 58r + p
        out[sl, :8 * TS, :] = og6.transpose(0, 1, 3, 2, 4, 5).reshape(
            PER_CORE, 8 * TS, W)
        ot5 = ot.reshape(PER_CORE // 4, 2, 2, 64, W)[:, :, :, :TAILM, :]
        # [tg, r, g, p, w] -> img 4tg + 2g + r, row 464 + p
        out[sl, 8 * TS:, :] = ot5.transpose(0, 2, 1, 3, 4).reshape(
            PER_CORE, TAILM, W)
    return out, res


def kernel(x, w):
    out, _ = _execute(x, w)
    return out
